# revision 9
# baseline (speedup 1.0000x reference)
"""Trainium2 Bass kernel for nn_CRFLayer (ragged sequence linear + token repack).

Reference computation:
    logits = embedding @ W.T + b            # [B, S, L]
    pack masked (mask==1) positions left per row -> [B, max_tok, L], zero pad
    pad_mask = arange(max_tok) < token_lens

Strategy (data parallel over batch, 2 rows / core on 8 cores):
  * Host computes gather indices from the mask (cheap metadata) and splits the
    fp32 embedding into bf16 hi + bf16 lo (hi+lo ~ fp32 to ~2^-17 rel).
  * Device gathers only the masked token rows straight into SBUF *transposed*
    (dma_gather transpose=True puts the contraction dim D on partitions), so
    the heavy HBM traffic is halved vs computing all S positions.
  * 16 accumulating bf16 matmuls per 128-token tile compute
    Ehi@[Whi|Wlo] and Elo@Whi into one PSUM tile; DVE folds the halves and a
    host-prebuilt masked bias (b at valid slots, 0 at pads).  Pad slots gather
    an all-zero row, so padding comes out exactly 0.
  * One batched DMA per core writes the packed [n_tiles*128, 32] result.
"""

import math

import numpy as np

B, S, D, L = 16, 2048, 1024, 32
N_CORES = 8
RPC = B // N_CORES  # batch rows per core
P = 128
DC = D // P  # contraction chunks of 128

# tokens per gather instruction (multiple of 128)
G = 128
# spread gathers across SWDGE queues
N_QUEUES = 1

TRACE = False
LAST = {}

_BUILD_CACHE = {}


def _build(n_t):
    """Build the Bass program for n_t 128-token tiles per batch row.

    Device inputs (per core):
      emb_hi, emb_lo: [RPC*S + 1, D] bf16, last row zero (pad target)
      idx:            [P, n_gathers * G/16] int16 gather indices (SWDGE wrap)
      wt:             [P, DC, 2L] bf16: wt[p,c,:L] = Whi[:,128c+p].T etc.
      bmask:          [P, n_tiles, L] f32: b at valid slots else 0
    Output: out [n_tiles*P, L] f32 (packed logits, row-major per tile).
    """
    import concourse.mybir as mybir
    import concourse.tile as tile
    from concourse import bacc, library_config

    n_tiles = RPC * n_t  # 128-token tiles per core
    n_gathers = n_tiles * P // G
    sub = G // P  # matmul tiles per gather
    n_rows_pad = RPC * S + 1
    gcols = G // 16  # idx columns per gather

    nc = bacc.Bacc(
        "TRN2",
        debug=False,
        enable_asserts=False,
        num_devices=N_CORES,
        num_swdge_queues=N_QUEUES,
    )
    bf16 = mybir.dt.bfloat16
    f32 = mybir.dt.float32

    emb_hi = nc.dram_tensor("emb_hi", [n_rows_pad, D], bf16, kind="ExternalInput")
    emb_lo = nc.dram_tensor("emb_lo", [n_rows_pad, D], bf16, kind="ExternalInput")
    idx = nc.dram_tensor("idx", [P, n_gathers * gcols], mybir.dt.int16, kind="ExternalInput")
    wt = nc.dram_tensor("wt", [P, DC, 2 * L], bf16, kind="ExternalInput")
    bmask = nc.dram_tensor("bmask", [P, n_tiles, L], f32, kind="ExternalInput")
    out = nc.dram_tensor("out", [n_tiles * P, L], f32, kind="ExternalOutput")

    with tile.TileContext(nc) as tc:
        with (
            tc.tile_pool(name="const", bufs=1) as cpool,
            tc.tile_pool(name="gat", bufs=4) as gpool,
            tc.tile_pool(name="ps", bufs=6, space="PSUM") as ppool,
            tc.tile_pool(name="outp", bufs=1) as opool,
        ):
            nc.gpsimd.load_library(library_config.mlp)

            idx_t = cpool.tile([P, n_gathers * gcols], mybir.dt.int16)
            nc.sync.dma_start(out=idx_t[:], in_=idx.ap())
            wt_t = cpool.tile([P, DC, 2 * L], bf16)
            nc.sync.dma_start(out=wt_t[:], in_=wt.ap())
            bm_t = cpool.tile([P, n_tiles, L], f32)
            nc.sync.dma_start(out=bm_t[:], in_=bmask.ap())
            out_t = opool.tile([P, n_tiles, L], f32)

            for g in range(n_gathers):
                ghi = gpool.tile([P, DC, G], bf16, tag="ghi")
                glo = gpool.tile([P, DC, G], bf16, tag="glo")
                islice = idx_t[:, g * gcols:(g + 1) * gcols]
                nc.gpsimd.dma_gather(
                    out_ap=ghi[:],
                    in_ap=emb_hi.ap(),
                    idxs_ap=islice,
                    num_idxs=G,
                    num_idxs_reg=G,
                    elem_size=D,
                    transpose=True,
                    queue_num=(2 * g) % N_QUEUES,
                )
                nc.gpsimd.dma_gather(
                    out_ap=glo[:],
                    in_ap=emb_lo.ap(),
                    idxs_ap=islice,
                    num_idxs=G,
                    num_idxs_reg=G,
                    elem_size=D,
                    transpose=True,
                    queue_num=(2 * g + 1) % N_QUEUES,
                )
                for m in range(sub):
                    i = g * sub + m  # global 128-token tile index
                    ms = slice(m * P, (m + 1) * P)
                    ps = ppool.tile([P, L], f32)
                    for c in range(DC):
                        # Ehi@Whi + Ehi@Wlo + Elo@Whi, all accumulated in PSUM
                        nc.tensor.matmul(
                            out=ps[:],
                            lhsT=ghi[:, c, ms],
                            rhs=wt_t[:, c, 0:L],
                            start=(c == 0),
                            stop=False,
                            skip_group_check=True,
                        )
                        nc.tensor.matmul(
                            out=ps[:],
                            lhsT=ghi[:, c, ms],
                            rhs=wt_t[:, c, L:2 * L],
                            start=False,
                            stop=False,
                            skip_group_check=True,
                        )
                        nc.tensor.matmul(
                            out=ps[:],
                            lhsT=glo[:, c, ms],
                            rhs=wt_t[:, c, 0:L],
                            start=False,
                            stop=(c == DC - 1),
                            skip_group_check=True,
                        )
                    nc.vector.tensor_add(
                        out=out_t[:, i, :], in0=ps[:], in1=bm_t[:, i, :]
                    )

            nc.sync.dma_start(
                out=out.ap().rearrange("(i p) l -> p i l", p=P),
                in_=out_t[:],
            )

    nc.compile()
    return nc


def _get_nc(n_t):
    if n_t not in _BUILD_CACHE:
        _BUILD_CACHE[n_t] = _build(n_t)
    return _BUILD_CACHE[n_t]


def _prep_inputs(embedding, mask, W, b):
    import ml_dtypes

    bf16 = ml_dtypes.bfloat16
    lens = mask.astype(np.int64).sum(axis=1)
    max_tok = int(lens.max())
    n_t = (max_tok + P - 1) // P
    n_tiles = RPC * n_t
    n_gathers = n_tiles * P // G
    gcols = G // 16
    npad = RPC * S  # index of the zero row

    emb2 = np.ascontiguousarray(embedding.reshape(B * S, D))
    ehi = emb2.astype(bf16)
    elo = (emb2 - ehi.astype(np.float32)).astype(bf16)

    Whi = W.astype(bf16)
    Wlo = (W - Whi.astype(np.float32)).astype(bf16)
    wt_host = np.empty((P, DC, 2 * L), dtype=bf16)
    wt_host[:, :, :L] = Whi.T.reshape(DC, P, L).transpose(1, 0, 2)
    wt_host[:, :, L:] = Wlo.T.reshape(DC, P, L).transpose(1, 0, 2)

    zrow = np.zeros((1, D), dtype=bf16)
    in_maps = []
    for k in range(N_CORES):
        r0 = k * RPC * S
        ehi_k = np.concatenate([ehi[r0:r0 + RPC * S], zrow], axis=0)
        elo_k = np.concatenate([elo[r0:r0 + RPC * S], zrow], axis=0)

        vals = np.full((n_tiles * P,), npad, dtype=np.int16)
        for r in range(RPC):
            pos = np.nonzero(mask[k * RPC + r])[0]
            lr = len(pos)
            o = r * n_t * P
            vals[o:o + lr] = (pos + r * S).astype(np.int16)
        valid = vals != npad
        # SWDGE idx wrap: slot j of gather g -> partition j%16, col g*gcols+j//16,
        # replicated into every 16-partition group (each Q7 cpu of the serving
        # queue reads its own group).
        idx16 = vals.reshape(n_gathers, gcols, 16).transpose(2, 0, 1).reshape(16, -1)
        idx_host = np.tile(idx16, (P // 16, 1))
        bm_host = (
            valid.reshape(n_tiles, P).T[:, :, None].astype(np.float32)
            * b.astype(np.float32)[None, None, :]
        )
        in_maps.append(
            dict(emb_hi=ehi_k, emb_lo=elo_k, idx=idx_host,
                 wt=wt_host, bmask=np.ascontiguousarray(bm_host))
        )
    return in_maps, lens, max_tok, n_t


def kernel(embedding, mask, W, b):
    from concourse.bass_utils import run_bass_kernel_spmd

    embedding = np.asarray(embedding, dtype=np.float32)
    mask = np.asarray(mask)
    W = np.asarray(W, dtype=np.float32)
    b = np.asarray(b, dtype=np.float32)

    in_maps, lens, max_tok, n_t = _prep_inputs(embedding, mask, W, b)
    nc = _get_nc(n_t)

    res = run_bass_kernel_spmd(
        nc, in_maps, core_ids=list(range(N_CORES)), trace=TRACE
    )
    LAST["results"] = res

    outs = [r["out"].reshape(RPC, n_t * P, L) for r in res.results]
    tok_logits = np.concatenate(outs, axis=0)[:, :max_tok, :].astype(np.float32)
    pad_mask = np.arange(max_tok)[None, :] < lens[:, None]
    return tok_logits, pad_mask
